# revision 1
# baseline (speedup 1.0000x reference)
"""Trainium2 Bass kernel for per-combination linear encoder (embedding lookup).

Computes z = y * w[idx] + b[idx] where idx = t*1024 + x @ [512,256,...,1]
for x in {0,1}^[N,10], t in {0,1}^[N,1], over a 2048-entry (w,b) table.

Sharding: data-parallel over the batch axis across 8 NeuronCores; the
tiny (w,b) table is replicated to every core (and every SBUF partition).

Per-core pipeline (tiles of [128 partitions x B rows], B per tile-schedule):
  1. DMA x/t/y tiles (contiguous per partition, p-major row assignment).
  2. DVE: idx = segmented-reduce(x * powers) + 1024*t, cast to int16.
  3. GPSIMD ap_gather (d=2) against a per-partition interleaved (w,b)
     table: out[p, c*16+q, :] = (w,b)[idx(16k+q, c)] for p in core k
     (each Q7 core gathers its 16 partitions' indices, wrapped order,
     output replicated across the core's partitions).
  4. TensorE un-wrap: 16 accumulating diagonal-mask matmuls per value
     pick og[p, c*16 + p%16] into compact PSUM tiles (exact: masks are
     0/1 so fp32 matmul selection is lossless).
  5. DVE FMA z = y*w + b, DMA out.

The gather dominates (~3.6 ns/row/core of GPSIMD time); all other
engines (DMA ~45us, DVE ~55us, PE ~290us) hide behind it.
"""

import numpy as np

import concourse.bacc as bacc
import concourse.mybir as mybir
from concourse.tile import TileContext
from concourse.bass_utils import run_bass_kernel_spmd

M = 8            # NeuronCores
P = 128          # SBUF partitions
# rows-per-partition schedule: sized to fit og (16*B*2 fp32) double-buffered
# in SBUF; small last tile shortens the post-gather tail (PE+FMA+store).
# RPP=1954 keeps batch padding minimal (N/8 = 250_000 -> 250_112 rows/core).
B_SCHED = (440, 440, 440, 440, 194)
RPP = sum(B_SCHED)          # rows per partition (1954)
R = P * RPP                 # rows per core (250_112)
D = 10           # covariate bits
C = 2048         # table entries
F32 = mybir.dt.float32
I16 = mybir.dt.int16

_CACHE = {}


def _build_program():
    nc = bacc.Bacc("TRN2", target_bir_lowering=False, debug=False, num_devices=M)

    x = nc.dram_tensor("x", [R, D], F32, kind="ExternalInput")
    t = nc.dram_tensor("t", [R], F32, kind="ExternalInput")
    y = nc.dram_tensor("y", [R], F32, kind="ExternalInput")
    wb = nc.dram_tensor("wb", [P, 2 * C], F32, kind="ExternalInput")
    pw = nc.dram_tensor("pw", [P, D], F32, kind="ExternalInput")
    mk = nc.dram_tensor("mk", [P, 16 * P], F32, kind="ExternalInput")
    z = nc.dram_tensor("z", [R], F32, kind="ExternalOutput")

    # row (tile i, partition p, col c) = (off_i*P + p*B_i + c) of the shard
    x3 = x.ap().rearrange("(pp r) d -> pp (r d)", pp=P)   # [P, RPP*D]
    t2 = t.ap().rearrange("(pp r) -> pp r", pp=P)          # [P, RPP]
    y2 = y.ap().rearrange("(pp r) -> pp r", pp=P)
    z2 = z.ap().rearrange("(pp r) -> pp r", pp=P)

    with TileContext(nc) as tc:
        with (
            tc.tile_pool(name="const", bufs=1) as cpool,
            tc.tile_pool(name="sb", bufs=2) as pool,
            tc.tile_pool(name="gat", bufs=2) as gpool,
            tc.tile_pool(name="ps", bufs=2, space="PSUM") as ppool,
        ):
            wb_t = cpool.tile([P, 2 * C], F32)
            nc.sync.dma_start(out=wb_t[:], in_=wb[:, :])
            pw_t = cpool.tile([P, D], F32)
            nc.sync.dma_start(out=pw_t[:], in_=pw[:, :])
            mk_t = cpool.tile([P, 16 * P], F32)
            nc.sync.dma_start(out=mk_t[:], in_=mk[:, :])

            off = 0
            for B in B_SCHED:
                xt = pool.tile([P, B * D], F32, tag="x")
                nc.sync.dma_start(out=xt[:], in_=x3[:, off * D:(off + B) * D])
                tt = pool.tile([P, B], F32, tag="t")
                nc.sync.dma_start(out=tt[:], in_=t2[:, off:off + B])
                yt = pool.tile([P, B], F32, tag="y")
                nc.sync.dma_start(out=yt[:], in_=y2[:, off:off + B])

                # x *= powers (in place; broadcast powers along the row dim)
                xv = xt[:].rearrange("p (b d) -> p b d", d=D)
                nc.vector.tensor_tensor(
                    out=xv, in0=xv,
                    in1=pw_t[:].unsqueeze(1).broadcast_to([P, B, D]),
                    op=mybir.AluOpType.mult,
                )
                # idx = sum_d x*2^(9-d)  (+ 1024*t below)
                idxf = pool.tile([P, B], F32, tag="idxf")
                nc.vector.tensor_reduce(
                    out=idxf[:], in_=xv, axis=mybir.AxisListType.X,
                    op=mybir.AluOpType.add,
                )
                t1024 = pool.tile([P, B], F32, tag="t1024")
                nc.vector.tensor_scalar_mul(out=t1024[:], in0=tt[:], scalar1=1024.0)
                nc.vector.tensor_tensor(
                    out=idxf[:], in0=idxf[:], in1=t1024[:], op=mybir.AluOpType.add
                )
                idx16 = pool.tile([P, B], I16, tag="idx16")
                nc.vector.tensor_copy(out=idx16[:], in_=idxf[:])

                # gather (w,b) pairs: og[p, c*16+q, :] = wb[idx(16k+q, c)]
                og = gpool.tile([P, 16 * B * 2], F32, tag="og")
                nc.gpsimd.ap_gather(
                    out_ap=og[:].rearrange("p (j e) -> p j e", e=2),
                    in_ap=wb_t[:].rearrange("p (c e) -> p c e", e=2),
                    idxs_ap=idx16[:],
                    channels=P, num_elems=C, d=2, num_idxs=16 * B,
                )

                # un-wrap via PE: psum[p, c] = sum_q 1[p%16==q] og[p, (c*16+q)*2+e]
                og3 = og[:].rearrange("p (c s) -> p c s", s=32)
                psw = ppool.tile([P, B], F32, tag="psw")
                psb = ppool.tile([P, B], F32, tag="psb")
                for q in range(16):
                    nc.tensor.matmul(
                        out=psw[:], lhsT=mk_t[:, q * P:(q + 1) * P],
                        rhs=og3[:, :, 2 * q], start=(q == 0), stop=(q == 15),
                    )
                for q in range(16):
                    nc.tensor.matmul(
                        out=psb[:], lhsT=mk_t[:, q * P:(q + 1) * P],
                        rhs=og3[:, :, 2 * q + 1], start=(q == 0), stop=(q == 15),
                    )

                # z = y*w + b
                zt = pool.tile([P, B], F32, tag="z")
                nc.vector.tensor_tensor(
                    out=zt[:], in0=yt[:], in1=psw[:], op=mybir.AluOpType.mult
                )
                nc.vector.tensor_tensor(
                    out=zt[:], in0=zt[:], in1=psb[:], op=mybir.AluOpType.add
                )
                nc.sync.dma_start(out=z2[:, off:off + B], in_=zt[:])
                off += B

    nc.compile()
    return nc


def _get_program():
    if "nc" not in _CACHE:
        _CACHE["nc"] = _build_program()
    return _CACHE["nc"]


def kernel(x, t, y, w, b, trace=False):
    N = x.shape[0]
    npad = M * R - N
    assert npad >= 0
    f32 = np.float32
    # rows assigned per (core, partition, col): shard row index
    # core m gets rows [m*R, (m+1)*R); within a core, partition p holds
    # rows [p*RPP, (p+1)*RPP) of its shard, contiguously.
    xp = np.concatenate([np.asarray(x, f32), np.zeros((npad, D), f32)]).reshape(M, R, D)
    tp = np.concatenate([np.asarray(t, f32).reshape(-1), np.zeros(npad, f32)]).reshape(M, R)
    yp = np.concatenate([np.asarray(y, f32).reshape(-1), np.zeros(npad, f32)]).reshape(M, R)
    wbi = np.empty(2 * C, f32)
    wbi[0::2] = np.asarray(w, f32)
    wbi[1::2] = np.asarray(b, f32)
    wb_rep = np.ascontiguousarray(np.tile(wbi[None, :], (P, 1)))
    pw_rep = np.ascontiguousarray(
        np.tile((2.0 ** np.arange(D - 1, -1, -1)).astype(f32)[None, :], (P, 1))
    )
    mk_host = np.zeros((P, 16 * P), f32)
    for k in range(P):
        mk_host[k, (k % 16) * P + k] = 1.0

    nc = _get_program()
    in_maps = [
        {"x": xp[i], "t": tp[i], "y": yp[i], "wb": wb_rep, "pw": pw_rep, "mk": mk_host}
        for i in range(M)
    ]
    res = run_bass_kernel_spmd(nc, in_maps, core_ids=list(range(M)), trace=trace)
    zfull = np.concatenate([res.results[i]["z"] for i in range(M)])[:N]
    out = zfull.reshape(N, 1).astype(np.float32)
    if trace:
        return out, res
    return out

